# revision 1
# baseline (speedup 1.0000x reference)
"""Trainium2 Bass kernel for nn_MultiHeadedAttention_44624710205499.

Reference computation (B=4, S=2048, D=512, H=8, dk=64, L=5):
  q = local_pool(query, 5)                    # causal 5-window softmax pooling
  k = local_pool(key @ W_fk + b_fk, 5)
  v = value @ W0 + b0
  x = MHA(q, k, v)   (full softmax, no mask)
  out = x @ Wout + bout

Sharding: 8 cores = (batch b = c//2) x (query-half = c%2).  Each core
computes 1024 query rows of one batch against all 2048 keys of that batch.
Gather on host is pure concatenation (+ transpose of the core's [D, SQ]
output layout).

On-device layout strategy: "feature-on-partition" (transposed) layout
throughout, which makes every step a natural matmul with no on-device
transposes:
  - scoresT[k, q] = kT-chunk.T @ qT          (lhsT=kT slice, rhs=qT slice)
  - eT = exp(scoresT)  (no max-subtraction: scores bounded ~30, fp32 safe)
  - x_augT[dv+1, q] = v_aug.T @ eT  accumulated over key chunks, where
    v_aug has a ones column => softmax denominator falls out as row 64.
  - divide by denominator after PV (softmax normalization is linear-safe)
  - outT = Wout-chunk.T @ xT  (host transposes the final [D, SQ] output)
Local pooling is banded attention over 516-row context windows with a
constant band mask; front zero-padding reproduces the reference's
zero-vector padding semantics exactly (score 0 -> weight exp(0)=1 in the
denominator, zero contribution to the numerator).

Matmul dtypes: float32r (1 cyc/row at N>=512 vs 4 for fp32) for all big
matmuls; walrus requires every producer of an f32r matmul operand to write
an f32r-declared tensor, so those tiles/DRAM tensors are declared float32r
(byte-identical to fp32 on the host side).  The pooling score matmuls use
bf16 operands: pooling is self-dominant (self score ~22.6 vs ~1 for
neighbors), so score-side quantization error cancels between numerator and
denominator, while the value-side path (kf_row/qrow) stays full precision.
"""

import math
import os

import ml_dtypes
import numpy as np

import concourse.bass as bass
import concourse.tile as tile
from concourse import bacc, mybir
from concourse import bass_utils

P = 128
B, S, D, H, DK, L = 4, 2048, 512, 8, 64, 5
SQ = S // 2            # query rows per core
NKI = D // P           # 4 contraction chunks of 128
SPAD = S + (L - 1)     # 2052 zero-front-padded kf length
SQPAD = SQ + (L - 1)   # 1028 query halo length
BLK = 512              # pooling block (positions per block)
NCH = 5                # ctx chunks per pooling block: 4x128 + 4
NBK = S // BLK         # 4 kf pooling blocks
NBQ = SQ // BLK        # 2 q pooling blocks
NQC = SQ // BLK        # 2 SDPA query chunks of 512
NKC = S // P           # 16 SDPA key chunks of 128
RSQD = 1.0 / math.sqrt(D)
RSQK = 1.0 / math.sqrt(DK)
NCORES = 8

F32 = mybir.dt.float32
BF16 = mybir.dt.bfloat16
F32R = mybir.dt.float32r

_PROG_CACHE = {}


def build_program(cfg=None):
    """Build + compile the per-core Bass program (same program on all 8 cores)."""
    cfg = dict(cfg or {})
    MDT = F32R if cfg.get("use_f32r", True) else F32   # big-matmul dtype
    sc_dt = BF16 if cfg.get("bf16_scores", True) else MDT

    def rd(ap):
        """read-view of an MDT tile for non-matmul (DVE/ACT) consumers."""
        return ap.bitcast(F32) if MDT == F32R else ap

    nc = bacc.Bacc(
        "TRN2",
        target_bir_lowering=False,
        debug=False,
        enable_asserts=False,
        num_devices=NCORES,
    )

    keyT_d = nc.dram_tensor("keyT", [D, SPAD], MDT, kind="ExternalInput").ap()
    valT_d = nc.dram_tensor("valT", [D, S], MDT, kind="ExternalInput").ap()
    qT_d = nc.dram_tensor("qT", [D, SQPAD], sc_dt, kind="ExternalInput").ap()
    qrow_d = nc.dram_tensor("qrow", [SQPAD, D], MDT, kind="ExternalInput").ap()
    wfk_d = nc.dram_tensor("wfk", [D, D], MDT, kind="ExternalInput").ap()
    w0_d = nc.dram_tensor("w0", [D, D], MDT, kind="ExternalInput").ap()
    wout_d = nc.dram_tensor("wout", [D, D], MDT, kind="ExternalInput").ap()
    ones_d = nc.dram_tensor("ones_col", [P, 1], MDT, kind="ExternalInput").ap()
    vones_d = nc.dram_tensor("vones", [P, 2, 8 * H], MDT, kind="ExternalInput").ap()
    bfk_col_d = nc.dram_tensor("bfk_col", [D, 1], F32, kind="ExternalInput").ap()
    bfk_row_d = nc.dram_tensor("bfk_row", [1, D], F32, kind="ExternalInput").ap()
    b0_row_d = nc.dram_tensor("b0_row", [1, D], F32, kind="ExternalInput").ap()
    bout_col_d = nc.dram_tensor("bout_col", [D, 1], F32, kind="ExternalInput").ap()
    mask_d = nc.dram_tensor("mask_band", [NCH * P, BLK], BF16, kind="ExternalInput").ap()
    kfpad_d = nc.dram_tensor("kfpad", [D, L - 1], sc_dt, kind="ExternalInput").ap()
    outT_d = nc.dram_tensor("outT", [D, SQ], F32, kind="ExternalOutput").ap()
    if cfg.get("dbg"):
        dbg_kTp_d = nc.dram_tensor("dbg_kTp", [D, S], F32, kind="ExternalOutput").ap()
        dbg_qTp_d = nc.dram_tensor("dbg_qTp", [D, SQ], F32, kind="ExternalOutput").ap()
        dbg_xt_d = nc.dram_tensor("dbg_xt", [D, SQ], F32, kind="ExternalOutput").ap()
        dbg_rec_d = nc.dram_tensor("dbg_rec", [H, BLK], F32, kind="ExternalOutput").ap()
        dbg_kf_d = nc.dram_tensor("dbg_kf", [D, S], sc_dt, kind="ExternalOutput").ap()

    with tile.TileContext(nc) as tc:
        with (
            tc.tile_pool(name="A", bufs=4) as pA,      # keyT -> valT -> outT
            tc.tile_pool(name="Bp", bufs=4) as pB,     # kfT -> xt4
            tc.tile_pool(name="C", bufs=2) as pC,      # kfr -> qT/qrow -> v
            tc.tile_pool(name="W", bufs=2) as pW,      # wfk -> w0 -> wout
            tc.tile_pool(name="kTp", bufs=4) as pK,
            tc.tile_pool(name="qTp", bufs=4) as pQ,
            tc.tile_pool(name="small", bufs=1) as pS,
            tc.tile_pool(name="esc", bufs=7) as pE,    # pooling exp tiles
            tc.tile_pool(name="esb", bufs=6) as pEb,   # SDPA exp tiles
            tc.tile_pool(name="rec", bufs=6) as pR,    # recips/broadcasts/tmp
            tc.tile_pool(name="psS", bufs=2, space="PSUM") as psS,
            tc.tile_pool(name="psV", bufs=3, space="PSUM") as psV,
            tc.tile_pool(name="psD", bufs=1, space="PSUM") as psD,
        ):
            # ---------------- constants / small loads ----------------
            mask_sb = pS.tile([P, NCH, BLK], BF16, tag="mask")
            nc.sync.dma_start(mask_sb[:], mask_d.rearrange("(m p) i -> p m i", p=P))
            bfk_col = pS.tile([P, NKI, 1], F32, tag="bfkc")
            nc.sync.dma_start(bfk_col[:], bfk_col_d.rearrange("(k p) o -> p k o", p=P))
            bout_col = pS.tile([P, NKI, 1], F32, tag="boutc")
            nc.sync.dma_start(bout_col[:], bout_col_d.rearrange("(k p) o -> p k o", p=P))
            ones_sb = pS.tile([P, 1], MDT, tag="ones")
            nc.sync.dma_start(ones_sb[:], ones_d[:])

            bfk_row = pR.tile([1, D], F32, tag="rec")
            nc.sync.dma_start(bfk_row[:], bfk_row_d[:])
            bfk_bc = pR.tile([P, D], F32, tag="rec")
            nc.gpsimd.partition_broadcast(bfk_bc[:], bfk_row[:])
            # variant with the 4 pad rows zeroed (for kf_row tile 0)
            bfk_bc0 = pS.tile([P, D], F32, tag="bfkbc0")
            nc.gpsimd.partition_broadcast(bfk_bc0[:], bfk_row[:])
            nc.vector.memset(bfk_bc0[0 : L - 1, :], 0.0)

            # ---------------- pooling (banded attention over 516-row ctx) -----
            def emit_pool(xT_slice, xrow, out_tiles, nblocks):
                """xT_slice(ki) -> [P, *PAD] transposed (padded) AP, sc_dt.
                xrow(n) -> (tile, idx) row-layout 128-row chunk n (padded rows).
                out_tiles: 4 x [P, nblocks*BLK] MDT pooled output (transposed)."""
                for t in range(nblocks):
                    es = []
                    for m in range(NCH):
                        K = P if m < NCH - 1 else L - 1
                        ps = psS.tile([P, BLK], F32, tag="psS")
                        for ki in range(NKI):
                            xa = xT_slice(ki)
                            nc.tensor.matmul(
                                ps[0:K, :],
                                xa[:, BLK * t + P * m : BLK * t + P * m + K],
                                xa[:, L - 1 + BLK * t : L - 1 + BLK * (t + 1)],
                                start=(ki == 0),
                                stop=(ki == NKI - 1),
                            )
                        e = pE.tile([P, BLK], MDT, tag="esc")
                        nc.scalar.activation(
                            e[0:K, :], ps[0:K, :],
                            mybir.ActivationFunctionType.Exp, scale=RSQD,
                        )
                        nc.vector.tensor_mul(e[0:K, :], rd(e[0:K, :]), mask_sb[0:K, m, :])
                        es.append(e)
                    dn = psD.tile([1, BLK], F32, tag="psD")
                    for m in range(NCH):
                        K = P if m < NCH - 1 else L - 1
                        nc.tensor.matmul(
                            dn[:],
                            ones_sb[0:K, :],
                            es[m][0:K, :],
                            start=(m == 0),
                            stop=(m == NCH - 1),
                        )
                    rc = pR.tile([1, BLK], F32, tag="rec")
                    nc.vector.reciprocal(rc[:], dn[:])
                    rb = pR.tile([P, BLK], F32, tag="rec")
                    nc.gpsimd.partition_broadcast(rb[:], rc[:])
                    for mo in range(NKI):
                        pv = psV.tile([P, BLK], F32, tag="psV", name=f"pv{mo}")
                        for m in range(NCH):
                            K = P if m < NCH - 1 else L - 1
                            t_, idx = xrow(4 * t + m)
                            nc.tensor.matmul(
                                pv[:],
                                t_[0:K, idx, P * mo : P * (mo + 1)],
                                es[m][0:K, :],
                                start=(m == 0),
                                stop=(m == NCH - 1),
                            )
                        nc.vector.tensor_mul(
                            out_tiles[mo][:, BLK * t : BLK * (t + 1)], pv[:], rb[:]
                        )


            PH = cfg.get("phases", 6)

            # ---------------- q loads + q pooling (independent of kf path) ----
            qT_all = pC.tile([P, NKI, SQPAD], sc_dt, tag="C")
            qT_r = qT_d.rearrange("(t p) s -> p t s", p=P)
            nc.sync.dma_start(qT_all[:, :, 0:516], qT_r[:, :, 0:516])
            nc.sync.dma_start(qT_all[:, :, 516:SQPAD], qT_r[:, :, 516:SQPAD])
            qrowA = pC.tile([P, 9, BLK], MDT, tag="C")
            qrow_r = qrow_d[0:SQ, :].rearrange("(n p) d -> p n d", p=P)
            nc.sync.dma_start(qrowA[:, 0:4, :], qrow_r[:, 0:4, :])
            nc.sync.dma_start(qrowA[:, 4:8, :], qrow_r[:, 4:8, :])
            nc.sync.dma_start(qrowA[0:4, 8, :], qrow_d[SQ:SQPAD, :])

            # ---------------- keyT + wfk loads (column-chunked for early start)
            keyT = [pA.tile([P, SPAD], MDT, tag="A", name=f"keyT{t}") for t in range(NKI)]
            bounds = [0, 516, 1028, 1540, SPAD]
            for cchunk in range(NKI):
                c0, c1 = bounds[cchunk], bounds[cchunk + 1]
                for t in range(NKI):
                    nc.sync.dma_start(keyT[t][:, c0:c1], keyT_d[P * t : P * (t + 1), c0:c1])
            wfk = pW.tile([P, NKI, D], MDT, tag="W")
            nc.sync.dma_start(wfk[:], wfk_d.rearrange("(k p) n -> p k n", p=P))

            qTp = [pQ.tile([P, SQ], MDT, tag="qTp", name=f"qTp{t}") for t in range(NKI)]
            if PH >= 3:
                emit_pool(lambda ki: qT_all[:, ki, :], lambda n: (qrowA, n), qTp, NBQ)

            # ---------------- kfT = (key @ W_fk + b_fk).T  [D, SPAD] ----------
            kfT = [pB.tile([P, SPAD], sc_dt, tag="B", name=f"kfT{t}") for t in range(NKI)]
            for mo in range(NKI):
                nc.sync.dma_start(kfT[mo][:, 0 : L - 1], kfpad_d[P * mo : P * (mo + 1), :])
            for ns in range(S // BLK):
                for mo in range(NKI):
                    ps = psS.tile([P, BLK], F32, tag="psS")
                    for ki in range(NKI):
                        nc.tensor.matmul(
                            ps[:],
                            wfk[:, ki, P * mo : P * (mo + 1)],
                            keyT[ki][:, L - 1 + BLK * ns : L - 1 + BLK * (ns + 1)],
                            start=(ki == 0),
                            stop=(ki == NKI - 1),
                        )
                    nc.scalar.add(
                        kfT[mo][:, L - 1 + BLK * ns : L - 1 + BLK * (ns + 1)],
                        ps[:],
                        bfk_col[:, mo, :],
                    )

            # ---------------- kf_row  [SPAD rows, D]  (17 x 128-row tiles) -----
            kfrA = pC.tile([P, 9, BLK], MDT, tag="C")
            kfrB = pC.tile([P, 8, BLK], MDT, tag="C")

            def kfr(n):
                return (kfrA, n) if n < 9 else (kfrB, n - 9)

            NROW = SPAD // P + 1  # 17
            for n in range(NROW):
                M = P if n < NROW - 1 else SPAD - P * (NROW - 1)  # 128 or 4
                ps = psS.tile([P, BLK], F32, tag="psS")
                for ki in range(NKI):
                    nc.tensor.matmul(
                        ps[0:M, :],
                        keyT[ki][:, P * n : P * n + M],
                        wfk[:, ki, :],
                        start=(ki == 0),
                        stop=(ki == NKI - 1),
                    )
                t_, idx = kfr(n)
                # tile 0 partitions 0:4 are the zero pad rows: psum rows are 0
                # there (zero key columns) and bfk_bc0 keeps them 0.
                bias = bfk_bc0 if n == 0 else bfk_bc
                nc.vector.tensor_add(t_[0:M, idx, :], ps[0:M, :], bias[0:M, :])

            kTp = [pK.tile([P, S], MDT, tag="kTp", name=f"kTp{t}") for t in range(NKI)]
            if PH >= 2:
                emit_pool(lambda ki: kfT[ki][:], kfr, kTp, NBK)

            # ---------------- v = value @ W0 + b0   [S rows, H, 65] -----------
            w0 = pW.tile([P, NKI, D], MDT, tag="W")
            nc.sync.dma_start(w0[:], w0_d.rearrange("(k p) n -> p k n", p=P))
            valT = [pA.tile([P, S], MDT, tag="A", name=f"valT{t}") for t in range(NKI)]
            for t in range(NKI):
                nc.sync.dma_start(valT[t][:], valT_d[P * t : P * (t + 1), :])
            b0_row = pR.tile([1, D], F32, tag="rec")
            nc.sync.dma_start(b0_row[:], b0_row_d[:])
            b0_bc = pR.tile([P, D], F32, tag="rec")
            nc.gpsimd.partition_broadcast(b0_bc[:], b0_row[:])

            vA = pC.tile([P, 8, H, DK + 1], MDT, tag="C")
            vB = pC.tile([P, 8, H, DK + 1], MDT, tag="C")
            nc.sync.dma_start(vA[:, :, :, DK], vones_d[:, 0, :].rearrange("p (n h) -> p n h", n=8))
            nc.sync.dma_start(vB[:, :, :, DK], vones_d[:, 1, :].rearrange("p (n h) -> p n h", n=8))
            for n in range(NKC if PH >= 4 else 0):
                ps = psS.tile([P, BLK], F32, tag="psS")
                for ki in range(NKI):
                    nc.tensor.matmul(
                        ps[:],
                        valT[ki][:, P * n : P * (n + 1)],
                        w0[:, ki, :],
                        start=(ki == 0),
                        stop=(ki == NKI - 1),
                    )
                vt = vA if n < 8 else vB
                nc.vector.tensor_add(
                    vt[:, n % 8, :, 0:DK],
                    ps[:].rearrange("p (h z) -> p h z", h=H),
                    b0_bc[:].rearrange("p (h z) -> p h z", h=H),
                )

            # ---------------- SDPA + output projection ----------------
            wout = pW.tile([P, NKI, D], MDT, tag="W")
            nc.sync.dma_start(wout[:], wout_d.rearrange("(k p) n -> p k n", p=P))
            xt4 = [pB.tile([P, SQ], MDT, tag="B", name=f"xt4_{t}") for t in range(NKI)]
            outT = [pA.tile([P, SQ], F32, tag="A", name=f"outT{t}") for t in range(NKI)]

            for h in range(H if PH >= 5 else 0):
                th, off = h // 2, DK * (h % 2)
                pxs = [psV.tile([DK + 1, BLK], F32, tag="psV", name=f"px{qc}")
                       for qc in range(NQC)]
                for kc in range(NKC):
                    ps = psS.tile([P, NQC * BLK], F32, tag="psS")
                    for qc in range(NQC):
                        nc.tensor.matmul(
                            ps[:, BLK * qc : BLK * (qc + 1)],
                            kTp[th][off : off + DK, P * kc : P * (kc + 1)],
                            qTp[th][off : off + DK, BLK * qc : BLK * (qc + 1)],
                            start=True,
                            stop=True,
                        )
                    e = pEb.tile([P, NQC * BLK], MDT, tag="esb")
                    nc.scalar.activation(
                        e[:], ps[:], mybir.ActivationFunctionType.Exp, scale=RSQK
                    )
                    vt = vA if kc < 8 else vB
                    for qc in range(NQC):
                        nc.tensor.matmul(
                            pxs[qc][:],
                            vt[:, kc % 8, h, :],
                            e[:, BLK * qc : BLK * (qc + 1)],
                            start=(kc == 0),
                            stop=(kc == NKC - 1),
                        )
                for qc in range(NQC):
                    px = pxs[qc]
                    # partition_broadcast HW ucode reads tile partition 0,
                    # not the AP base -- reciprocal at base 64 (lane-aligned),
                    # then DMA the row down to a base-0 tile for the broadcast.
                    rc = pR.tile([DK + 1, BLK], F32, tag="rec")
                    nc.vector.reciprocal(rc[DK : DK + 1, :], px[DK : DK + 1, :])
                    rc0 = pR.tile([1, BLK], F32, tag="rec")
                    nc.sync.dma_start(rc0[:], rc[DK : DK + 1, :])
                    rb = pR.tile([DK, BLK], F32, tag="rec")
                    nc.gpsimd.partition_broadcast(rb[:], rc0[:])
                    if cfg.get("dbg") and qc == 0:
                        nc.sync.dma_start(dbg_rec_d[h : h + 1, :], rb[0:1, :])
                    if h % 2 == 0:
                        nc.vector.tensor_mul(
                            xt4[th][0:DK, BLK * qc : BLK * (qc + 1)], px[0:DK, :], rb[:]
                        )
                    else:
                        tmp = pR.tile([DK, BLK], MDT, tag="rec")
                        nc.vector.tensor_mul(tmp[:], px[0:DK, :], rb[:])
                        nc.sync.dma_start(
                            xt4[th][DK:P, BLK * qc : BLK * (qc + 1)], tmp[:]
                        )
            for qc in range(NQC if PH >= 6 else 0):
                for mo in range(NKI):
                    po = psS.tile([P, BLK], F32, tag="psS")
                    for ki in range(NKI):
                        nc.tensor.matmul(
                            po[:],
                            wout[:, ki, P * mo : P * (mo + 1)],
                            xt4[ki][:, BLK * qc : BLK * (qc + 1)],
                            start=(ki == 0),
                            stop=(ki == NKI - 1),
                        )
                    nc.scalar.add(
                        outT[mo][:, BLK * qc : BLK * (qc + 1)], po[:], bout_col[:, mo, :]
                    )
                    nc.sync.dma_start(
                        outT_d[P * mo : P * (mo + 1), BLK * qc : BLK * (qc + 1)],
                        outT[mo][:, BLK * qc : BLK * (qc + 1)],
                    )
            if cfg.get("dbg"):
                for t in range(NKI):
                    nc.sync.dma_start(dbg_kTp_d[P * t : P * (t + 1), :],
                                      rd(kTp[t][:]))
                    nc.sync.dma_start(dbg_qTp_d[P * t : P * (t + 1), :],
                                      rd(qTp[t][:]))
                    nc.sync.dma_start(dbg_xt_d[P * t : P * (t + 1), :],
                                      rd(xt4[t][:]))
                    nc.sync.dma_start(dbg_kf_d[P * t : P * (t + 1), :],
                                      kfT[t][:, L - 1 :])

    nc.compile()
    return nc


def make_band_mask():
    j = np.arange(NCH * P)[:, None]
    i = np.arange(BLK)[None, :]
    return (((j - i) >= 0) & ((j - i) <= L - 1)).astype(np.float32)


def make_core_inputs(query, key, value, W_fk, b_fk, W0, b0, Wout, bout, cfg=None):
    """Build the 8 per-core input dicts from full inputs (host-side shard)."""
    cfg = dict(cfg or {})
    bf16_scores = cfg.get("bf16_scores", True)
    sc_np = ml_dtypes.bfloat16 if bf16_scores else np.float32
    shared = {
        "wfk": np.ascontiguousarray(W_fk, np.float32),
        "w0": np.ascontiguousarray(W0, np.float32),
        "wout": np.ascontiguousarray(Wout, np.float32),
        "ones_col": np.ones((P, 1), np.float32),
        "vones": np.ones((P, 2, 8 * H), np.float32),
        "bfk_col": np.ascontiguousarray(b_fk.reshape(D, 1), np.float32),
        "bfk_row": np.ascontiguousarray(b_fk.reshape(1, D), np.float32),
        "b0_row": np.ascontiguousarray(b0.reshape(1, D), np.float32),
        "bout_col": np.ascontiguousarray(bout.reshape(D, 1), np.float32),
        "mask_band": make_band_mask().astype(ml_dtypes.bfloat16),
        "kfpad": np.zeros((D, L - 1), sc_np),
    }
    in_maps = []
    for c in range(NCORES):
        b, half = divmod(c, 2)
        q0 = half * SQ
        q_halo = np.zeros((SQPAD, D), np.float32)
        lo = max(0, q0 - (L - 1))
        q_halo[(L - 1) - (q0 - lo):] = query[b, lo : q0 + SQ]
        keyT_pad = np.zeros((D, SPAD), np.float32)
        keyT_pad[:, L - 1 :] = key[b].T
        m = dict(shared)
        m["keyT"] = keyT_pad
        m["valT"] = np.ascontiguousarray(value[b].T, np.float32)
        m["qT"] = np.ascontiguousarray(q_halo.T).astype(sc_np)
        m["qrow"] = q_halo
        in_maps.append(m)
    return in_maps


def _cfg_from_env():
    cfg_key = os.environ.get("ATT_KERNEL_CFG", "")
    cfg = {}
    if "no_f32r" in cfg_key:
        cfg["use_f32r"] = False
    if "f32_scores" in cfg_key:
        cfg["bf16_scores"] = False
    return cfg


def get_program(cfg=None):
    cfg = dict(cfg or {})
    key_t = tuple(sorted(cfg.items()))
    if key_t not in _PROG_CACHE:
        _PROG_CACHE[key_t] = build_program(cfg)
    return _PROG_CACHE[key_t]


def kernel(query, key, value, mask=None, W_fk=None, b_fk=None, W0=None, b0=None,
           Wout=None, bout=None, **extra):
    del mask, extra  # mask is dead in the reference (forward passes mask=None)
    cfg = _cfg_from_env()
    nc = get_program(cfg)

    query = np.asarray(query, np.float32)
    key = np.asarray(key, np.float32)
    value = np.asarray(value, np.float32)
    in_maps = make_core_inputs(
        query, key, value,
        np.asarray(W_fk, np.float32), np.asarray(b_fk, np.float32),
        np.asarray(W0, np.float32), np.asarray(b0, np.float32),
        np.asarray(Wout, np.float32), np.asarray(bout, np.float32),
        cfg,
    )
    res = bass_utils.run_bass_kernel_spmd(nc, in_maps, core_ids=list(range(NCORES)))
    out = np.empty((B, S, D), np.float32)
    for c in range(NCORES):
        b, half = divmod(c, 2)
        out[b, half * SQ : (half + 1) * SQ, :] = res.results[c]["outT"].T
    return out



# revision 2
# speedup vs baseline: 34.5677x; 34.5677x over previous
"""Trainium2 Bass kernel v2 for nn_MultiHeadedAttention_44624710205499.

Reference computation (B=4, S=2048, D=512, H=8, dk=64, L=5):
  q = local_pool(query, 5)                    # causal 5-window softmax pooling
  k = local_pool(key @ W_fk + b_fk, 5)
  v = value @ W0 + b0
  x = MHA(q, k, v)   (full softmax, no mask)
  out = x @ Wout + bout

Sharding: 8 cores = (batch b = c//2) x (query-half = c%2).  Each core
computes 1024 query rows of one batch against all 2048 keys of that batch.

v2 structure (vs baseline):
  - bf16 operands everywhere (PSUM accumulation stays f32).
  - pooling: 128-position blocks, causal 132-row context = 128-row main +
    4-row spill (the band corner only touches the last 4 block columns, so
    spill matmuls run at the N<=64 floor).  Scores/exp/mask are batched 4
    blocks per psum tile (fewer, larger ACT instructions); denominators are
    software-pipelined behind the sweep; PV is mo-major into batched psum
    tiles with the normalization folded into the DVE eviction multiply.
  - kf_row comes from PE transposes of kfT (batched evictions on Pool)
    instead of recomputing key @ W_fk in row layout.
  - kfT matmuls interleave into the q-pool sweep; transposes + the
    v-projection into the kf-pool sweep; kf-PV for D-chunk mo=th+1 is
    emitted inside SDPA pair th's loop (engines are free under the
    ACT-bound pairs) - filling stall slots of chain-bound phases.
  - SDPA: head-pair score matmuls at base partitions 0/64 (row groups
    (0,0)/(64,0)) => concurrent on the two 64-row halves of the PE array.
    Moving dim 1024 (bf16).  PV keeps the ones-column denominator trick.
  - SDPA epilogue: 1/denominator is broadcast across partitions with a
    K=1 f32r matmul into PSUM (no DMA round-trip); only the odd head's
    xt4 half needs one sbuf->sbuf DMA per (pair, qc) to cross partitions.
  - ACT does the softmax exps (the binding resource) plus half the kfT
    bias adds and the output bias; DMA issue count is minimized.
  - cfg["reps"]: emit the body R times (same tiles => serialized) for
    replay-difference device timing.
"""

import math
import os

import ml_dtypes
import numpy as np

import concourse.bass as bass
import concourse.tile as tile
from concourse import bacc, mybir
from concourse import bass_utils

P = 128
B, S, D, H, DK, L = 4, 2048, 512, 8, 64, 5
SQ = S // 2            # query rows per core
NKI = D // P           # 4 contraction chunks of 128
SPAD = S + (L - 1)     # 2052 zero-front-padded kf length
SQPAD = SQ + (L - 1)   # 1028 query halo length
PB = 128               # pooling block (positions per block)
CTX = PB + (L - 1)     # 132 context rows per pooling block
GB = 4                 # pooling blocks batched per psum/exp tile
NBK = S // PB          # 16 kf pooling blocks
NBQ = SQ // PB         # 8 q pooling blocks
NKC = S // P           # 16 SDPA key chunks of 128
NTH = H // 2           # 4 head pairs
RSQD = 1.0 / math.sqrt(D)
RSQK = 1.0 / math.sqrt(DK)
NCORES = 8

F32 = mybir.dt.float32
F32R = mybir.dt.float32r
BF16 = mybir.dt.bfloat16
EXPF = mybir.ActivationFunctionType.Exp

_PROG_CACHE = {}


def build_program(cfg=None):
    """Build + compile the per-core Bass program (same program on all 8 cores)."""
    cfg = dict(cfg or {})
    REPS = int(cfg.get("reps", 1))

    nc = bacc.Bacc(
        "TRN2",
        target_bir_lowering=False,
        debug=False,
        enable_asserts=False,
        num_devices=NCORES,
    )

    keyT_d = nc.dram_tensor("keyT", [D, S], BF16, kind="ExternalInput").ap()
    valT_d = nc.dram_tensor("valT", [D, S], BF16, kind="ExternalInput").ap()
    qT_d = nc.dram_tensor("qT", [D, SQPAD], BF16, kind="ExternalInput").ap()
    qrow_d = nc.dram_tensor("qrow", [SQPAD, D], BF16, kind="ExternalInput").ap()
    wfk_d = nc.dram_tensor("wfk", [D, D], BF16, kind="ExternalInput").ap()
    w0_d = nc.dram_tensor("w0", [D, D], BF16, kind="ExternalInput").ap()
    wout_d = nc.dram_tensor("wout", [D, D], BF16, kind="ExternalInput").ap()
    # packed constants (2 DMAs):
    #   cbf16 [P, 658]: ident(128) | mask4(512) | ones(1) | pad(1) | mask2x4(16)
    #   cf32  [P, 9]:   bfk_col(4) | bout_col(4) | pad(1)
    cbf_d = nc.dram_tensor("cbf16", [P, 658], BF16, kind="ExternalInput").ap()
    cf32_d = nc.dram_tensor("cf32", [P, 9], F32, kind="ExternalInput").ap()
    onesr_d = nc.dram_tensor("onesr", [P, DK], F32R, kind="ExternalInput").ap()
    outT_d = nc.dram_tensor("outT", [D, SQ], F32, kind="ExternalOutput").ap()

    with tile.TileContext(nc) as tc:
        with (
            tc.tile_pool(name="A", bufs=4) as pA,      # keyT -> valT
            tc.tile_pool(name="Bp", bufs=4) as pB,     # kfT -> xt4
            tc.tile_pool(name="C", bufs=4) as pC,      # qT,qrow,kfrA,kfrB -> v
            tc.tile_pool(name="W", bufs=2) as pW,      # wfk -> w0 -> wout
            tc.tile_pool(name="kTp", bufs=4) as pK,
            tc.tile_pool(name="qTp", bufs=4) as pQ,
            tc.tile_pool(name="small", bufs=1) as pS,
            tc.tile_pool(name="eq", bufs=5) as pEq,    # q-pool exp tiles
            tc.tile_pool(name="ek", bufs=9) as pEk,    # kf-pool exp tiles
            tc.tile_pool(name="esb", bufs=4) as pEb,   # SDPA exp tiles
            tc.tile_pool(name="rec", bufs=6) as pR,    # recips/tmp
            tc.tile_pool(name="out", bufs=2) as pO,    # outT staging
            tc.tile_pool(name="psS", bufs=2, space="PSUM") as psS,  # 2x4KB
            tc.tile_pool(name="psV", bufs=4, space="PSUM") as psV,  # 4x2KB
        ):
            # ---------------- constants (loaded once) ----------------
            cbf = pS.tile([P, 658], BF16, tag="cbf")
            nc.sync.dma_start(cbf[:], cbf_d[:])
            ident = cbf[:, 0:P]
            mask4 = cbf[:, P : P + 512]
            ones_sb = cbf[:, 640:641]
            mask2x4 = cbf[0 : L - 1, 642:658]
            cf = pS.tile([P, 9], F32, tag="cf32")
            nc.sync.dma_start(cf[:], cf32_d[:])
            bfk_col = cf[:, 0:NKI]
            bout_col = cf[:, NKI : 2 * NKI]
            onesr = pS.tile([P, DK], F32R, tag="onesr")
            nc.sync.dma_start(onesr[:], onesr_d[:])

            for rep in range(REPS):
                # ---------- input loads: few, large, spread over queues ----
                qT_all = pC.tile([P, NKI, SQPAD], BF16, tag="C", name="qT")
                qT_r = qT_d.rearrange("(t p) s -> p t s", p=P)
                nc.scalar.dma_start(qT_all[:, 0:2, :], qT_r[:, 0:2, :])
                nc.gpsimd.dma_start(qT_all[:, 2:4, :], qT_r[:, 2:4, :])
                qrA = pC.tile([P, 9, D], BF16, tag="C", name="qr")
                qrow_r = qrow_d[0:SQ, :].rearrange("(n p) d -> p n d", p=P)
                nc.gpsimd.dma_start(qrA[:, 0:8, :], qrow_r[:])
                nc.gpsimd.dma_start(qrA[0 : L - 1, 8, :], qrow_d[SQ:SQPAD, :])

                keyT = [pA.tile([P, S], BF16, tag="A", name=f"keyT{t}")
                        for t in range(NKI)]
                nc.sync.dma_start(keyT[0][:], keyT_d[0:P, :])
                nc.scalar.dma_start(keyT[1][:], keyT_d[P : 2 * P, :])
                nc.sync.dma_start(keyT[2][:], keyT_d[2 * P : 3 * P, :])
                nc.scalar.dma_start(keyT[3][:], keyT_d[3 * P : 4 * P, :])
                wfk = pW.tile([P, NKI, D], BF16, tag="W", name="wfk")
                nc.sync.dma_start(wfk[:], wfk_d.rearrange("(k p) n -> p k n", p=P))
                valT = [pA.tile([P, S], BF16, tag="A", name=f"valT{t}")
                        for t in range(NKI)]
                for t in range(NKI):
                    nc.gpsimd.dma_start(valT[t][:], valT_d[P * t : P * (t + 1), :])
                w0 = pW.tile([P, NKI, D], BF16, tag="W", name="w0")
                nc.gpsimd.dma_start(w0[:], w0_d.rearrange("(k p) n -> p k n", p=P))

                kfT = [pB.tile([P, SPAD], BF16, tag="B", name=f"kfT{t}")
                       for t in range(NKI)]
                for mo in range(NKI):
                    nc.vector.memset(kfT[mo][:, 0 : L - 1], 0.0)

                def emit_kft(ns, mo):
                    ps = psS.tile([P, 512], F32, tag="psS", name="kfm")
                    for ki in range(NKI):
                        nc.tensor.matmul(
                            ps[:],
                            wfk[:, ki, P * mo : P * (mo + 1)],
                            keyT[ki][:, 512 * ns : 512 * (ns + 1)],
                            start=(ki == 0),
                            stop=(ki == NKI - 1),
                        )
                    dst = kfT[mo][:, L - 1 + 512 * ns : L - 1 + 512 * (ns + 1)]
                    nc.scalar.add(dst, ps[:], bfk_col[:, mo : mo + 1])

                def pool_scores_grp(xT, g, pE, scale):
                    """scores+exp+mask for pooling blocks 4g..4g+3, batched
                    into one [P, 512] psum + one [4, 16] spill psum."""
                    ps4 = psV.tile([P, GB * PB], F32, tag="psV", name="plm")
                    ps2 = psS.tile([L - 1, GB * 4], F32, tag="psS", name="pls")
                    for i in range(GB):
                        t = GB * g + i
                        for ki in range(NKI):
                            xa = xT(ki)
                            nc.tensor.matmul(
                                ps4[:, PB * i : PB * (i + 1)],
                                xa[:, PB * t : PB * t + P],
                                xa[:, PB * t + 4 : PB * t + 4 + PB],
                                start=(ki == 0), stop=(ki == NKI - 1),
                                skip_group_check=True,
                            )
                            nc.tensor.matmul(
                                ps2[:, 4 * i : 4 * (i + 1)],
                                xa[:, PB * t + P : PB * t + CTX],
                                xa[:, PB * t + P : PB * t + CTX],
                                start=(ki == 0), stop=(ki == NKI - 1),
                                skip_group_check=True,
                            )
                    e4 = pE.tile([P, GB * PB], BF16, tag="e", name="e4")
                    nc.scalar.activation(e4[:], ps4[:], EXPF, scale=scale)
                    nc.vector.tensor_mul(e4[:], e4[:], mask4)
                    e2 = pE.tile([L - 1, GB * 4], BF16, tag="e", name="e2")
                    nc.scalar.activation(e2[:], ps2[:], EXPF, scale=scale)
                    nc.vector.tensor_mul(e2[:], e2[:], mask2x4)
                    return e4, e2

                def pool_den(e4, e2, i, rc_all, t):
                    pd = psS.tile([1, PB], F32, tag="psS", name="pld")
                    nc.tensor.matmul(pd[:], ones_sb, e4[:, PB * i : PB * (i + 1)],
                                     start=True, stop=True)
                    nc.tensor.matmul(pd[0:1, PB - 4 : PB],
                                     ones_sb[0 : L - 1, :],
                                     e2[0 : L - 1, 4 * i : 4 * (i + 1)],
                                     start=False, stop=True,
                                     skip_group_check=True)
                    nc.vector.reciprocal(rc_all[0:1, PB * t : PB * (t + 1)], pd[:])

                def pool_pv_grp(xr, g, mo, e4, e2, out_tile, rb_all, pool_pp):
                    """normalized PV for blocks 4g..4g+3, D-chunk mo: batched
                    psum + one DVE eviction multiply (normalize-at-evict)."""
                    pp, ptag = pool_pp
                    pv = pp.tile([P, GB * PB], F32, tag=ptag, name="plv")
                    for i in range(GB):
                        t = GB * g + i
                        tm, im = xr(t)
                        ts, isp = xr(t + 1)
                        nc.tensor.matmul(
                            pv[:, PB * i : PB * (i + 1)],
                            tm[:, im, P * mo : P * (mo + 1)],
                            e4[:, PB * i : PB * (i + 1)],
                            start=True, stop=True, skip_group_check=True,
                        )
                        nc.tensor.matmul(
                            pv[:, PB * (i + 1) - 4 : PB * (i + 1)],
                            ts[0 : L - 1, isp, P * mo : P * (mo + 1)],
                            e2[0 : L - 1, 4 * i : 4 * (i + 1)],
                            start=False, stop=True, skip_group_check=True,
                        )
                    c0 = GB * PB * g
                    nc.vector.tensor_mul(
                        out_tile[:, c0 : c0 + GB * PB], pv[:],
                        rb_all[:, c0 : c0 + GB * PB],
                    )

                # ============ phase A: q-pool sweep x kfT ============
                rcq = pS.tile([1, SQ], F32, tag="rcq")
                qes = []
                NGQ = NBQ // GB  # 2
                for g in range(NGQ):
                    qes.append(pool_scores_grp(
                        lambda ki: qT_all[:, ki, :], g, pEq, RSQD))
                for u in range(16):
                    emit_kft(u // NKI, u % NKI)
                    if u % 2 == 0:
                        g, i = divmod(u // 2, GB)
                        pool_den(*qes[g], i, rcq, u // 2)
                rbq = pS.tile([P, SQ], F32, tag="rbq")
                nc.gpsimd.partition_broadcast(rbq[:], rcq[:])
                qTp = [pQ.tile([P, SQ], BF16, tag="qTp", name=f"qTp{t}")
                       for t in range(NKI)]
                for mo in range(NKI):
                    for g in range(NGQ):
                        e4, e2 = qes[g]
                        pool_pv_grp(lambda r: (qrA, r), g, mo, e4, e2,
                                    qTp[mo], rbq, (psV, "psV"))

                # ============ phase B: kf-pool sweep x transposes x v ======
                kfrA = pC.tile([P, 9, D], BF16, tag="C", name="kfrA")
                kfrB = pC.tile([P, 8, D], BF16, tag="C", name="kfrB")

                def kfr(r):
                    return (kfrA, r) if r < 9 else (kfrB, r - 9)

                NROW = SPAD // P + 1  # 17
                def emit_transp(rg):
                    """transpose row-chunks 4rg..4rg+3, one batched Pool
                    eviction per mo (full 128-row chunks only)."""
                    rr = list(range(4 * rg, 4 * rg + 4))
                    for mo in range(NKI):
                        pt = psV.tile([P, 4 * P], BF16, tag="psV", name="ptr")
                        with nc.allow_low_precision(reason="transpose, no accumulation"):
                            for j, r in enumerate(rr):
                                nc.tensor.transpose(
                                    pt[:, P * j : P * (j + 1)],
                                    kfT[mo][:, P * r : P * (r + 1)], ident
                                )
                        t_, idx = kfr(rr[0])
                        if rr[-1] < 9 or rr[0] >= 9:
                            dst = t_[:, idx : idx + 4, P * mo : P * (mo + 1)]
                            nc.vector.tensor_copy(
                                dst, pt[:].rearrange("p (n z) -> p n z", z=P))
                        else:  # straddles the kfrA/kfrB boundary (rg=2: r 8..11)
                            nsp = 9 - rr[0]
                            nc.vector.tensor_copy(
                                t_[:, idx : idx + nsp, P * mo : P * (mo + 1)],
                                pt[:, 0 : nsp * P].rearrange("p (n z) -> p n z", z=P))
                            t2, idx2 = kfr(rr[nsp])
                            nc.vector.tensor_copy(
                                t2[:, idx2 : idx2 + 4 - nsp,
                                   P * mo : P * (mo + 1)],
                                pt[:, nsp * P : 4 * P].rearrange(
                                    "p (n z) -> p n z", z=P))

                def emit_transp_tail():
                    """transpose the 4-row tail chunk (padded rows 2048:2052)."""
                    MT = SPAD - P * (NROW - 1)  # 4
                    for mo in range(NKI):
                        pt = psV.tile([P, P], BF16, tag="psV", name="ptt")
                        with nc.allow_low_precision(reason="transpose, no accumulation"):
                            nc.tensor.transpose(
                                pt[0:MT, :],
                                kfT[mo][:, P * (NROW - 1) : SPAD], ident)
                        t_, idx = kfr(NROW - 1)
                        nc.vector.tensor_copy(
                            t_[0:MT, idx, P * mo : P * (mo + 1)], pt[0:MT, :])

                # ---- v tiles (pC ring: reuse qT_all/qrA buffers) ----
                vA = pC.tile([P, 8, H, DK + 1], BF16, tag="C", name="vA")
                vB = pC.tile([P, 8, H, DK + 1], BF16, tag="C", name="vB")
                nc.vector.memset(vA[:, :, :, DK], 1.0)
                nc.vector.memset(vB[:, :, :, DK], 1.0)

                def emit_v(n):
                    ps = psS.tile([P, 512], F32, tag="psS", name="vm")
                    for ki in range(NKI):
                        nc.tensor.matmul(
                            ps[:],
                            valT[ki][:, P * n : P * (n + 1)],
                            w0[:, ki, :],
                            start=(ki == 0),
                            stop=(ki == NKI - 1),
                        )
                    vt = vA if n < 8 else vB
                    nc.scalar.copy(
                        vt[:, n % 8, :, 0:DK],
                        ps[:].rearrange("p (h z) -> p h z", h=H),
                    )

                rck = pS.tile([1, S], F32, tag="rck")
                kes = []
                NGK = NBK // GB  # 4
                for g in range(NGK):
                    kes.append(pool_scores_grp(
                        lambda ki: kfT[ki][:], g, pEk, RSQD))
                    emit_transp(g)
                    for n in range(4):
                        emit_v(4 * g + n)
                    for i in range(GB):
                        pool_den(*kes[g], i, rck, GB * g + i)
                emit_transp_tail()
                rbk = pS.tile([P, S], F32, tag="rbk")
                nc.gpsimd.partition_broadcast(rbk[:], rck[:])
                kTp = [pK.tile([P, S], BF16, tag="kTp", name=f"kTp{t}")
                       for t in range(NKI)]
                # mo=0 now; mo=1..3 are emitted inside SDPA pairs 0..2
                for g in range(NGK):
                    e4, e2 = kes[g]
                    pool_pv_grp(kfr, g, 0, e4, e2, kTp[0], rbk, (psV, "psV"))

                # ============ phase C: SDPA ============
                wout = pW.tile([P, NKI, D], BF16, tag="W", name="wout")
                nc.sync.dma_start(wout[:], wout_d.rearrange("(k p) n -> p k n", p=P))
                xt4 = [pB.tile([P, SQ], BF16, tag="B", name=f"xt4_{t}")
                       for t in range(NKI)]

                for th in range(NTH):
                    hA, hB = 2 * th, 2 * th + 1
                    pxA = [psV.tile([DK + 1, 512], F32, tag="psV", name=f"pxA{qc}")
                           for qc in range(2)]
                    pxB = [psV.tile([DK + 1, 512], F32, tag="psV", name=f"pxB{qc}")
                           for qc in range(2)]
                    for kc in range(NKC):
                        psa = psS.tile([P, SQ], F32, tag="psS", name="sca")
                        psb = psS.tile([P, SQ], F32, tag="psS", name="scb")
                        # row-tiled pair: even head on array rows 0:64,
                        # odd head on rows 64:128 -> concurrent on HW.
                        # (one matmul per 512-col slice: a matmul output
                        # cannot cross the 2KB psum bank boundary)
                        for qc in range(2):
                            nc.tensor.matmul(
                                psa[:, 512 * qc : 512 * (qc + 1)],
                                kTp[th][0:DK, P * kc : P * (kc + 1)],
                                qTp[th][0:DK, 512 * qc : 512 * (qc + 1)],
                                start=True, stop=True, skip_group_check=True,
                            )
                            nc.tensor.matmul(
                                psb[:, 512 * qc : 512 * (qc + 1)],
                                kTp[th][DK:P, P * kc : P * (kc + 1)],
                                qTp[th][DK:P, 512 * qc : 512 * (qc + 1)],
                                start=True, stop=True, skip_group_check=True,
                            )
                        eA = pEb.tile([P, SQ], BF16, tag="esb", name="eA")
                        nc.scalar.activation(eA[:], psa[:], EXPF, scale=RSQK)
                        eB = pEb.tile([P, SQ], BF16, tag="esb", name="eB")
                        nc.scalar.activation(eB[:], psb[:], EXPF, scale=RSQK)
                        vt = vA if kc < 8 else vB
                        for qc in range(2):
                            nc.tensor.matmul(
                                pxA[qc][:],
                                vt[:, kc % 8, hA, :],
                                eA[:, 512 * qc : 512 * (qc + 1)],
                                start=(kc == 0), stop=(kc == NKC - 1),
                            )
                            nc.tensor.matmul(
                                pxB[qc][:],
                                vt[:, kc % 8, hB, :],
                                eB[:, 512 * qc : 512 * (qc + 1)],
                                start=(kc == 0), stop=(kc == NKC - 1),
                            )
                        # hide kf-PV mo=th+1 inside pair th's slack (th<3)
                        if th < NTH - 1 and kc % 4 == 3:
                            g, mo = kc // 4, th + 1
                            e4, e2 = kes[g]
                            pool_pv_grp(kfr, g, mo, e4, e2,
                                        kTp[mo], rbk, (psS, "psS"))
                    # epilogue in px ring order (A0, A1, B0, B1) so the next
                    # pair's accumulators free up as early as possible
                    for pxs, half in ((pxA, 0), (pxB, 1)):
                        for qc in range(2):
                            px = pxs[qc]
                            # reciprocal of the ones-column (row 64), then
                            # broadcast down partitions with a K=1 f32r matmul
                            # (no DMA round-trip; tile_position (64, 0)).
                            rc = pR.tile([DK + 1, 512], F32R, tag="rec")
                            with nc.allow_low_precision(
                                    reason="f32r is bit-identical to f32"):
                                nc.vector.reciprocal(rc[DK : DK + 1, :],
                                                     px[DK : DK + 1, :])
                            rb = psS.tile([DK, 512], F32, tag="psS", name="rb")
                            nc.tensor.matmul(rb[:], onesr[DK : DK + 1, :],
                                             rc[DK : DK + 1, :],
                                             start=True, stop=True)
                            # DVE may read only one PSUM operand per op
                            rbs = pR.tile([DK, 512], F32, tag="rec")
                            nc.vector.tensor_copy(rbs[:], rb[:])
                            dst = xt4[th][DK * half : DK * (half + 1),
                                          512 * qc : 512 * (qc + 1)]
                            if half == 0:
                                nc.vector.tensor_mul(dst, px[0:DK, :], rbs[:])
                            else:
                                tmp = pR.tile([DK, 512], BF16, tag="rec")
                                nc.vector.tensor_mul(tmp[:], px[0:DK, :], rbs[:])
                                nc.gpsimd.dma_start(dst, tmp[:])

                # ---------------- phase D: output projection ----------------
                for mo in range(NKI):
                    ot = pO.tile([P, SQ], F32, tag="out")
                    for qc in range(2):
                        pp, ptag = (psS, "psS") if (2 * mo + qc) % 2 == 0 \
                            else (psV, "psV")
                        po = pp.tile([P, 512], F32, tag=ptag, name="op")
                        for ki in range(NKI):
                            nc.tensor.matmul(
                                po[:],
                                wout[:, ki, P * mo : P * (mo + 1)],
                                xt4[ki][:, 512 * qc : 512 * (qc + 1)],
                                start=(ki == 0),
                                stop=(ki == NKI - 1),
                            )
                        nc.scalar.add(ot[:, 512 * qc : 512 * (qc + 1)], po[:],
                                      bout_col[:, mo : mo + 1])
                    nc.sync.dma_start(outT_d[P * mo : P * (mo + 1), :], ot[:])

    nc.compile()
    return nc


def make_core_inputs(query, key, value, W_fk, b_fk, W0, b0, Wout, bout, cfg=None):
    """Build the 8 per-core input dicts from full inputs (host-side shard)."""
    bf = ml_dtypes.bfloat16
    j = np.arange(P)[:, None]
    i = np.arange(P)[None, :]
    mask1 = ((j - i >= 0) & (j - i <= L - 1)).astype(np.float32)
    mask4 = np.tile(mask1, (1, GB))
    m2 = (np.arange(L - 1)[:, None] <= np.arange(L - 1)[None, :]).astype(np.float32)
    mask2x4 = np.zeros((P, GB * 4), np.float32)
    mask2x4[0 : L - 1, :] = np.tile(m2, (1, GB))
    cbf = np.concatenate(
        [np.eye(P, dtype=np.float32), mask4,
         np.ones((P, 1), np.float32), np.zeros((P, 1), np.float32), mask2x4],
        axis=1).astype(bf)
    bout_fold = (b0.astype(np.float64) @ Wout.astype(np.float64)
                 + bout.astype(np.float64)).astype(np.float32)
    cf32 = np.concatenate(
        [b_fk.reshape(NKI, P).T, bout_fold.reshape(NKI, P).T,
         np.zeros((P, 1), np.float32)], axis=1).astype(np.float32)
    shared = {
        "wfk": np.ascontiguousarray(W_fk).astype(bf),
        "w0": np.ascontiguousarray(W0).astype(bf),
        "wout": np.ascontiguousarray(Wout).astype(bf),
        "cbf16": cbf,
        "cf32": np.ascontiguousarray(cf32),
        "onesr": np.ones((P, DK), np.float32),
    }
    in_maps = []
    for c in range(NCORES):
        b, half = divmod(c, 2)
        q0 = half * SQ
        q_halo = np.zeros((SQPAD, D), np.float32)
        lo = max(0, q0 - (L - 1))
        q_halo[(L - 1) - (q0 - lo):] = query[b, lo : q0 + SQ]
        m = dict(shared)
        m["keyT"] = np.ascontiguousarray(key[b].T).astype(bf)
        m["valT"] = np.ascontiguousarray(value[b].T).astype(bf)
        m["qT"] = np.ascontiguousarray(q_halo.T).astype(bf)
        m["qrow"] = q_halo.astype(bf)
        in_maps.append(m)
    return in_maps


def _cfg_from_env():
    cfg = {}
    r = os.environ.get("ATT_KERNEL_REPS", "")
    if r:
        cfg["reps"] = int(r)
    return cfg


def get_program(cfg=None):
    cfg = dict(cfg or {})
    key_t = tuple(sorted(cfg.items()))
    if key_t not in _PROG_CACHE:
        _PROG_CACHE[key_t] = build_program(cfg)
    return _PROG_CACHE[key_t]


def kernel(query, key, value, mask=None, W_fk=None, b_fk=None, W0=None, b0=None,
           Wout=None, bout=None, **extra):
    del mask, extra  # mask is dead in the reference (forward passes mask=None)
    cfg = _cfg_from_env()
    nc = get_program(cfg)

    query = np.asarray(query, np.float32)
    key = np.asarray(key, np.float32)
    value = np.asarray(value, np.float32)
    in_maps = make_core_inputs(
        query, key, value,
        np.asarray(W_fk, np.float32), np.asarray(b_fk, np.float32),
        np.asarray(W0, np.float32), np.asarray(b0, np.float32),
        np.asarray(Wout, np.float32), np.asarray(bout, np.float32),
        cfg,
    )
    res = bass_utils.run_bass_kernel_spmd(nc, in_maps, core_ids=list(range(NCORES)))
    out = np.empty((B, S, D), np.float32)
    for c in range(NCORES):
        b, half = divmod(c, 2)
        out[b, half * SQ : (half + 1) * SQ, :] = res.results[c]["outT"].T
    return out
